# revision 12
# baseline (speedup 1.0000x reference)
"""TransformerConv 2-layer GNN encoder on 8 Trainium2 NeuronCores (Bass/Tile).

v2 strategy (graph-partition parallel, bf16 on-chip):
  - Nodes padded 50000 -> 50176 = 8 cores x 49 tiles x 128. Each core owns 49
    consecutive node tiles as TARGETS; edges are assigned to the core owning
    their dst, grouped per dst-tile, and within each tile split into a
    lo-half (src < 25088) and hi-half group so table indices fit int16 for
    dma_gather. Groups are padded to 128-edge chunks; chunk counts are
    equalized across cores so the SPMD program is identical.
  - Layer 1 k|v table: every core computes the full [50176,256] bf16 table
    from host-pretransposed xT (no device transposes, no collective);
    q and the skip projection are computed for local tiles only (q -> small
    DRAM table for per-edge gather, skip stays in SBUF).
  - Edge phase per tile: 3 dma_gather instructions (kv-lo, kv-hi, q) fetch
    all per-edge rows for the tile's ~18 chunks in one shot each. Chunks are
    processed in pairs: eps = ea@We on PE, kj|vj = gather + eps via one
    broadcast add, one-hot S built by is_equal vs iota, per-edge logits via
    mult+per-head reduce, exp on ACT, weighted values, segment-softmax
    accumulated in PSUM by S^T @ rhs matmuls. Node update fuses 1/sum,
    skip add, leaky-relu.
  - Layer 2 tables: h tiles transposed on PE as they are produced, written to
    a [1,128,NLC] hT slab; ONE AllGather moves hT to all cores, which then
    replicate the kv2 table compute. q2/s2 are local (no collective needed).
Softmax note: segment-max subtraction is skipped (alphas are O(1); exact
softmax invariance) and the divide is applied after summation, matching the
reference up to rounding. On-chip data is bf16; accumulation is fp32 in PSUM.
Padding slots gather row 0 (bounded junk) and are zeroed by S's empty column.
"""
import numpy as np

P = 128
N = 50000
NP_ = 50176
TILES = 392
NCORES = 8
TPC = TILES // NCORES          # 49 tiles per core
NLC = TPC * P                  # 6272 local nodes
HID = 128
ED = 16
DSTREL_PAD = 200.0
SLABE = 64                     # ea chunks per slab DMA


def _bf16():
    import ml_dtypes
    return np.dtype(ml_dtypes.bfloat16)


def _wrap16(arr):
    """int16 flat index list -> [128, len/16] wrapped+replicated layout."""
    w = arr.reshape(-1, 16).T.astype(np.int16)        # [16, len/16]
    return np.tile(w, (8, 1))                          # [128, len/16]


# ----------------------------------------------------------------- host prep
def _prep(ei, ea):
    bf16 = _bf16()
    LO = NP_ // 2
    src = np.asarray(ei[0], dtype=np.int64)
    dst = np.asarray(ei[1], dtype=np.int64)
    ea = np.asarray(ea, dtype=np.float32)

    order = np.argsort(dst, kind="stable")
    src_s, dst_s, ea_s = src[order], dst[order], ea[order]

    tile_of = dst_s // P
    tile_starts = np.searchsorted(tile_of, np.arange(TILES))
    tile_ends = np.searchsorted(tile_of, np.arange(TILES), side="right")

    # per (core, tile): lo/hi edge lists
    lohi = [[None] * TPC for _ in range(NCORES)]
    nlo = np.zeros((NCORES, TPC), np.int64)
    nhi = np.zeros((NCORES, TPC), np.int64)
    for c in range(NCORES):
        for tl in range(TPC):
            tg = c * TPC + tl
            a, b = tile_starts[tg], tile_ends[tg]
            s, d, e = src_s[a:b], dst_s[a:b], ea_s[a:b]
            m = s < LO
            lohi[c][tl] = (s[m], d[m], e[m], s[~m] - LO, d[~m], e[~m])
            nlo[c, tl], nhi[c, tl] = m.sum(), (~m).sum()

    Clo = np.maximum(-(-nlo.max(axis=0) // P), 1)      # [TPC]
    Chi = np.maximum(-(-nhi.max(axis=0) // P), 1)
    Cloc = Clo + Chi
    NCH = int(Cloc.sum())
    off = np.zeros(TPC, dtype=np.int64)
    off[1:] = np.cumsum(Cloc)[:-1]

    cores = []
    for c in range(NCORES):
        nslot = NCH * P
        drel_sl = np.full(nslot, DSTREL_PAD, dtype=np.float32)
        ea_sl = np.zeros((nslot, ED), dtype=np.float32)
        kv_idx = np.zeros(nslot, dtype=np.int16)
        q_idx = np.zeros(nslot, dtype=np.int16)
        for tl in range(TPC):
            slo, dlo, elo, shi, dhi, ehi = lohi[c][tl]
            tg = c * TPC + tl
            s0 = off[tl] * P                       # lo slots begin
            s1 = (off[tl] + Clo[tl]) * P           # hi slots begin
            nl, nh = len(slo), len(shi)
            drel_sl[s0:s0 + nl] = (dlo - tg * P).astype(np.float32)
            drel_sl[s1:s1 + nh] = (dhi - tg * P).astype(np.float32)
            ea_sl[s0:s0 + nl] = elo
            ea_sl[s1:s1 + nh] = ehi
            kv_idx[s0:s0 + nl] = slo
            kv_idx[s1:s1 + nh] = shi
            q_idx[s0:s0 + nl] = dlo - c * NLC
            q_idx[s1:s1 + nh] = dhi - c * NLC
        idx16 = np.concatenate([_wrap16(kv_idx), _wrap16(q_idx)], axis=1)
        cores.append(dict(
            idx16=np.ascontiguousarray(idx16),
            drelT=np.ascontiguousarray(
                drel_sl.reshape(NCH, P).T.astype(bf16)),
            eaT=np.ascontiguousarray(ea_sl.T.astype(bf16)),
        ))
    return cores, Clo, Chi, off, NCH


# ------------------------------------------------------- walrus wait legalize
def _legalize_waits(nc):
    import concourse.mybir as mybir
    k = 0
    for bb in nc.main_func.blocks:
        il = bb.instructions
        new = []
        for ins in il:
            si = ins.sync_info
            if si is not None and len(si.on_wait) > 1:
                waits = list(si.on_wait)
                for w in waits[:-1]:
                    nop = mybir.InstNoOp(name=f"wn{k}-{ins.name}", ins=[], outs=[])
                    k += 1
                    nop.engine = ins.engine
                    nop.sync_info = mybir.SyncInfo(on_wait=[w], on_update=[])
                    new.append(nop)
                ins.sync_info = mybir.SyncInfo(on_wait=[waits[-1]],
                                               on_update=list(si.on_update))
            new.append(ins)
        il[:] = new


# ------------------------------------------------------------- device program
def _build(Clo, Chi, off, NCH, stop_after=None):
    import concourse.bass as bass
    import concourse.mybir as mybir
    import concourse.tile as tile
    from concourse import library_config
    f32 = mybir.dt.float32
    bf = mybir.dt.bfloat16
    i16 = mybir.dt.int16
    Alu = mybir.AluOpType
    Act = mybir.ActivationFunctionType
    Cloc = Clo + Chi
    LO = NP_ // 2

    nc = bass.Bass()
    xT_full = nc.declare_dram_parameter("xT_full", [P, NP_], bf, isOutput=False)
    xT_loc = nc.declare_dram_parameter("xT_loc", [P, NLC], bf, isOutput=False)
    idx16 = nc.declare_dram_parameter("idx16", [P, NCH * 16], i16,
                                      isOutput=False)
    drelT = nc.declare_dram_parameter("drelT", [P, NCH], bf, isOutput=False)
    eaT = nc.declare_dram_parameter("eaT", [ED, NCH * P], bf, isOutput=False)
    iotaP = nc.declare_dram_parameter("iotaP", [P, P], bf, isOutput=False)
    identP = nc.declare_dram_parameter("identP", [P, P], f32, isOutput=False)
    Wkv1 = nc.declare_dram_parameter("Wkv1", [HID, 256], bf, isOutput=False)
    Wqs1 = nc.declare_dram_parameter("Wqs1", [HID, 256], bf, isOutput=False)
    We1 = nc.declare_dram_parameter("We1", [ED, HID], bf, isOutput=False)
    Wkv2 = nc.declare_dram_parameter("Wkv2", [HID, 256], bf, isOutput=False)
    Wqs2 = nc.declare_dram_parameter("Wqs2", [HID, 256], bf, isOutput=False)
    We2 = nc.declare_dram_parameter("We2", [ED, HID], bf, isOutput=False)
    out = nc.declare_dram_parameter("out", [NLC, HID], f32, isOutput=True)

    kv1_tab = nc.dram_tensor("kv1_tab", [NP_, 256], bf, kind="Internal")
    q1_loc = nc.dram_tensor("q1_loc", [NLC, HID], bf, kind="Internal")
    hT_loc = nc.dram_tensor("hT_loc", [1, P, NLC], bf, kind="Internal")
    hT_full = nc.dram_tensor("hT_full", [NCORES, P, NLC], bf, kind="Internal")
    kv2_tab = nc.dram_tensor("kv2_tab", [NP_, 256], bf, kind="Internal")
    q2_loc = nc.dram_tensor("q2_loc", [NLC, HID], bf, kind="Internal")

    nc.gpsimd.load_library(library_config.mlp)

    _nregs = {}

    def nreg(v):
        if v not in _nregs:
            _nregs[v] = nc.gpsimd.to_reg(v)
        return _nregs[v]

    with tile.TileContext(nc) as tc:
        with tc.tile_pool(name="cst", bufs=1) as cst, \
             tc.tile_pool(name="xp", bufs=3) as xp, \
             tc.tile_pool(name="evp", bufs=3) as evp, \
             tc.tile_pool(name="slb", bufs=2) as slb, \
             tc.tile_pool(name="kvp", bufs=2) as kvp, \
             tc.tile_pool(name="qgp", bufs=2) as qgp, \
             tc.tile_pool(name="wp", bufs=3) as wp, \
             tc.tile_pool(name="np_", bufs=3) as np_, \
             tc.tile_pool(name="pskv", bufs=2, space="PSUM") as pskv, \
             tc.tile_pool(name="pse", bufs=2, space="PSUM") as pse, \
             tc.tile_pool(name="psa", bufs=2, space="PSUM") as psa, \
             tc.tile_pool(name="pst", bufs=2, space="PSUM") as pst:

            # ---------------- constants / persistent state
            iota_b = cst.tile([P, P], bf)
            nc.sync.dma_start(out=iota_b[:], in_=iotaP[:])
            ident_f = cst.tile([P, P], f32)
            nc.sync.dma_start(out=ident_f[:], in_=identP[:])

            wkv1 = cst.tile([HID, 256], bf)
            nc.sync.dma_start(out=wkv1[:], in_=Wkv1[:])
            wqs1 = cst.tile([HID, 256], bf)
            nc.sync.dma_start(out=wqs1[:], in_=Wqs1[:])
            we1 = cst.tile([ED, HID], bf)
            nc.sync.dma_start(out=we1[:], in_=We1[:])
            wkv2 = cst.tile([HID, 256], bf)
            nc.sync.dma_start(out=wkv2[:], in_=Wkv2[:])
            wqs2 = cst.tile([HID, 256], bf)
            nc.sync.dma_start(out=wqs2[:], in_=Wqs2[:])
            we2 = cst.tile([ED, HID], bf)
            nc.sync.dma_start(out=we2[:], in_=We2[:])

            idx_sb = cst.tile([P, NCH * 16], i16)
            nc.sync.dma_start(out=idx_sb[:], in_=idx16[:])
            drelT_sb = cst.tile([P, NCH], bf)
            nc.sync.dma_start(out=drelT_sb[:], in_=drelT[:])

            s1_sb = cst.tile([P, NLC], bf)
            s2_sb = cst.tile([P, NLC], bf)

            # ---------------- table build: full kv + local q/s
            def tab_phase(kv_tab, q_loc, s_sb, wkv, wqs, xin_full, xin_loc):
                for gt in range(TILES):
                    xt = xp.tile([P, P], bf, tag="xt")
                    if xin_full is not None:
                        nc.sync.dma_start(out=xt[:],
                                          in_=xin_full[:, gt * P:(gt + 1) * P])
                    else:
                        nc.sync.dma_start(
                            out=xt[:],
                            in_=hT_full[gt // TPC, :,
                                        (gt % TPC) * P:((gt % TPC) + 1) * P])
                    ps = pskv.tile([P, 256], f32, space="PSUM", tag="ps")
                    nc.tensor.matmul(out=ps[:], lhsT=xt[:], rhs=wkv[:],
                                     start=True, stop=True)
                    ev = evp.tile([P, 256], bf, tag="ev")
                    nc.scalar.activation(out=ev[:], in_=ps[:], func=Act.Copy)
                    nc.sync.dma_start(out=kv_tab[gt * P:(gt + 1) * P, :],
                                      in_=ev[:])
                for tl in range(TPC):
                    xt = xp.tile([P, P], bf, tag="xt")
                    if xin_loc is not None:
                        nc.sync.dma_start(out=xt[:],
                                          in_=xin_loc[:, tl * P:(tl + 1) * P])
                    else:
                        nc.sync.dma_start(
                            out=xt[:], in_=hT_loc[0, :, tl * P:(tl + 1) * P])
                    ps = pskv.tile([P, 256], f32, space="PSUM", tag="ps")
                    nc.tensor.matmul(out=ps[:], lhsT=xt[:], rhs=wqs[:],
                                     start=True, stop=True)
                    qv = evp.tile([P, 256], bf, tag="ev")
                    nc.scalar.activation(out=qv[:], in_=ps[:], func=Act.Copy)
                    nc.sync.dma_start(out=q_loc[tl * P:(tl + 1) * P, :],
                                      in_=qv[:, 0:HID])
                    nc.vector.tensor_copy(out=s_sb[:, tl * P:(tl + 1) * P],
                                          in_=qv[:, HID:256])

            # ---------------- edge phase
            def edge_phase(kv_tab, q_loc, s_sb, we, heads, layer1,
                           gathers_only=False):
                D = HID // heads
                W = HID + heads
                scale = 1.0 / float(np.sqrt(D))
                NCHMAX = int(Cloc.max())
                slabs = {}

                def ea_slab(c):
                    sb_i = c // SLABE
                    if sb_i not in slabs:
                        c0 = sb_i * SLABE
                        c1 = min(NCH, c0 + SLABE)
                        t = slb.tile([ED, SLABE * P], bf, tag="ea")
                        nc.sync.dma_start(out=t[:, :(c1 - c0) * P],
                                          in_=eaT[:, c0 * P:c1 * P])
                        slabs[sb_i] = t
                    return slabs[sb_i], (c - (c // SLABE) * SLABE) * P

                for tl in range(TPC):
                    clo, chi = int(Clo[tl]), int(Chi[tl])
                    nch_t = clo + chi
                    c0 = int(off[tl])
                    # dma_gather crashes above ~1024 idxs/instruction
                    # (SWDGE descriptor ring limit); split into <=8-chunk
                    # pieces.
                    CMAX = 8

                    def gath(dst, dst_ch0, n_ch, src_ap, idx_ch0, elem):
                        for b in range(0, n_ch, CMAX):
                            g = min(CMAX, n_ch - b)
                            nc.gpsimd.dma_gather(
                                out_ap=dst[:, (dst_ch0 + b) * elem:
                                           (dst_ch0 + b + g) * elem].rearrange(
                                    "p (c d) -> p c d", d=elem),
                                in_ap=src_ap,
                                idxs_ap=idx_sb[:, (idx_ch0 + b) * 8:
                                               (idx_ch0 + b + g) * 8],
                                num_idxs=g * P, num_idxs_reg=nreg(g * P),
                                elem_size=elem)

                    kvg = kvp.tile([P, NCHMAX * 256], bf, tag="kvg")
                    gath(kvg, 0, clo, kv_tab[0:LO, :], c0, 256)
                    gath(kvg, clo, chi, kv_tab[LO:NP_, :], c0 + clo, 256)
                    qg = qgp.tile([P, NCHMAX * HID], bf, tag="qg")
                    gath(qg, 0, nch_t, q_loc[:], NCH + c0, HID)
                    if gathers_only:
                        og = np_.tile([P, HID], f32, tag="og")
                        nc.vector.tensor_tensor(out=og[:], in0=kvg[:, 0:HID],
                                                in1=qg[:, 0:HID], op=Alu.add)
                        nc.sync.dma_start(out=out[tl * P:(tl + 1) * P, :],
                                          in_=og[:])
                        continue
                    acc = psa.tile([P, W], f32, space="PSUM", tag="acc")

                    for k0 in range(0, nch_t, 2):
                        G = min(2, nch_t - k0)
                        c = c0 + k0
                        eps_ps = pse.tile([P, 2 * HID], f32, space="PSUM",
                                          tag="eps")
                        for g in range(G):
                            sl, col = ea_slab(c + g)
                            nc.tensor.matmul(
                                out=eps_ps[:, g * HID:(g + 1) * HID],
                                lhsT=sl[:, col:col + P], rhs=we[:],
                                start=True, stop=True)
                        esb = wp.tile([P, 2 * HID], bf, tag="esb")
                        nc.scalar.activation(out=esb[:, :G * HID],
                                             in_=eps_ps[:, :G * HID],
                                             func=Act.Copy)
                        kvj = wp.tile([P, 2 * 256], bf, tag="kvj")
                        nc.vector.tensor_tensor(
                            out=kvj[:, :G * 256].rearrange(
                                "p (g t d) -> p g t d", g=G, t=2),
                            in0=kvg[:, k0 * 256:(k0 + G) * 256].rearrange(
                                "p (g t d) -> p g t d", g=G, t=2),
                            in1=esb[:, :G * HID].rearrange(
                                "p (g d) -> p g () d", g=G)
                                .to_broadcast([P, G, 2, HID]),
                            op=Alu.add)
                        S2 = wp.tile([P, 2 * P], bf, tag="S2")
                        nc.vector.tensor_tensor(
                            out=S2[:, :G * P].rearrange("p (g n) -> p g n", g=G),
                            in0=drelT_sb[:, c:c + G].rearrange("p g -> p g ()")
                                .to_broadcast([P, G, P]),
                            in1=iota_b[:].rearrange("p n -> p () n")
                                .to_broadcast([P, G, P]),
                            op=Alu.is_equal)
                        kvj3 = kvj[:, :G * 256].rearrange(
                            "p (g d2) -> p g d2", d2=256)
                        prod = wp.tile([P, 2 * HID], bf, tag="prod")
                        nc.vector.tensor_tensor(
                            out=prod[:, :G * HID].rearrange(
                                "p (g d) -> p g d", g=G),
                            in0=kvj3[:, :, 0:HID],
                            in1=qg[:, k0 * HID:(k0 + G) * HID].rearrange(
                                "p (g d) -> p g d", g=G),
                            op=Alu.mult)
                        alpha = np_.tile([P, 2 * heads], f32, tag="alpha")
                        nc.vector.tensor_reduce(
                            out=alpha[:, :G * heads],
                            in_=prod[:, :G * HID].rearrange(
                                "p (gh d) -> p gh d", d=D),
                            axis=mybir.AxisListType.X, op=Alu.add)
                        rhs2 = wp.tile([P, 2 * W], bf, tag="rhs2")
                        r3 = rhs2[:, :G * W].rearrange("p (g w) -> p g w", g=G)
                        nc.scalar.activation(
                            out=r3[:, :, HID:W],
                            in_=alpha[:, :G * heads].rearrange(
                                "p (g h) -> p g h", g=G),
                            func=Act.Exp, scale=scale)
                        if heads == 1:
                            nc.vector.tensor_tensor(
                                out=r3[:, :, 0:HID],
                                in0=kvj3[:, :, HID:256],
                                in1=r3[:, :, HID:W].to_broadcast([P, G, HID]),
                                op=Alu.mult)
                        else:
                            nc.vector.tensor_tensor(
                                out=r3[:, :, 0:HID].rearrange(
                                    "p g (h d) -> p g h d", h=heads),
                                in0=kvj3[:, :, HID:256].rearrange(
                                    "p g (h d) -> p g h d", h=heads),
                                in1=r3[:, :, HID:W].rearrange(
                                    "p g h -> p g h ()")
                                    .to_broadcast([P, G, heads, D]),
                                op=Alu.mult)
                        for g in range(G):
                            nc.tensor.matmul(
                                out=acc[:], lhsT=S2[:, g * P:(g + 1) * P],
                                rhs=r3[:, g, :],
                                start=(k0 + g == 0),
                                stop=(k0 + g == nch_t - 1))

                    # ---- node update for tile tl
                    den = np_.tile([P, heads], f32, tag="den")
                    nc.vector.tensor_scalar_add(out=den[:],
                                                in0=acc[:, HID:W],
                                                scalar1=1e-16)
                    rinv = np_.tile([P, heads], f32, tag="rinv")
                    nc.vector.reciprocal(out=rinv[:], in_=den[:])
                    attn = np_.tile([P, HID], f32, tag="attn")
                    if heads == 1:
                        nc.vector.tensor_scalar_mul(out=attn[:],
                                                    in0=acc[:, 0:HID],
                                                    scalar1=rinv[:, 0:1])
                    else:
                        nc.vector.tensor_tensor(
                            out=attn[:].rearrange("p (h d) -> p h d", h=heads),
                            in0=acc[:, 0:HID].rearrange("p (h d) -> p h d",
                                                        h=heads),
                            in1=rinv[:].rearrange("p h -> p h ()")
                                .to_broadcast([P, heads, D]),
                            op=Alu.mult)
                    ht = np_.tile([P, HID], f32, tag="ht")
                    nc.vector.tensor_tensor(out=ht[:], in0=attn[:],
                                            in1=s_sb[:, tl * P:(tl + 1) * P],
                                            op=Alu.add)
                    if layer1:
                        # leaky_relu(x) = max(0.01*x, x)
                        hf = np_.tile([P, HID], f32, tag="hf")
                        nc.vector.scalar_tensor_tensor(
                            out=hf[:], in0=ht[:], scalar=0.01, in1=ht[:],
                            op0=Alu.mult, op1=Alu.max)
                        tps = pst.tile([P, P], f32, space="PSUM", tag="tps")
                        nc.tensor.transpose(out=tps[:], in_=hf[:],
                                            identity=ident_f[:])
                        hT = np_.tile([P, P], bf, tag="hT")
                        nc.scalar.activation(out=hT[:], in_=tps[:],
                                             func=Act.Copy)
                        nc.sync.dma_start(out=hT_loc[0, :, tl * P:(tl + 1) * P],
                                          in_=hT[:])
                        if stop_after == "edge1":
                            nc.sync.dma_start(
                                out=out[tl * P:(tl + 1) * P, :], in_=ht[:])
                    else:
                        nc.sync.dma_start(out=out[tl * P:(tl + 1) * P, :],
                                          in_=ht[:])

            tab_phase(kv1_tab, q1_loc, s1_sb, wkv1, wqs1, xT_full, xT_loc)
            if stop_after == "tab1":
                ot = evp.tile([P, HID], f32, tag="ot")
                nc.vector.tensor_copy(out=ot[:], in_=s1_sb[:, 0:HID])
                nc.sync.dma_start(out=out[0:P, :], in_=ot[:])
            else:
                edge_phase(kv1_tab, q1_loc, s1_sb, we1, heads=8, layer1=True,
                           gathers_only=(stop_after == "gathers"))
                if stop_after not in ("edge1", "gathers"):
                    nc.gpsimd.collective_compute(
                        "AllGather", mybir.AluOpType.bypass,
                        replica_groups=[list(range(NCORES))],
                        ins=[hT_loc[:].opt()], outs=[hT_full[:].opt()])
                    if stop_after == "ag":
                        ot = evp.tile([P, HID], f32, tag="ot")
                        nc.vector.tensor_copy(out=ot[:], in_=s1_sb[:, 0:HID])
                        nc.sync.dma_start(out=out[0:P, :], in_=ot[:])
                    else:
                        tab_phase(kv2_tab, q2_loc, s2_sb, wkv2, wqs2, None, None)
                        edge_phase(kv2_tab, q2_loc, s2_sb, we2, heads=1,
                                   layer1=False)

    _legalize_waits(nc)
    import concourse.mybir as _mb
    _mb.codegen_inst_isa_subclasses(nc)
    return nc


_CACHE = {}


def kernel(x, ei, ea, Wq1, bq1, Wk1, bk1, Wv1, bv1, We1, Ws1, bs1,
           Wq2, bq2, Wk2, bk2, Wv2, bv2, We2, Ws2, bs2):
    from concourse.bass_utils import run_bass_kernel_spmd
    bf16 = _bf16()

    for b in (bq1, bk1, bv1, bs1, bq2, bk2, bv2, bs2):
        assert not np.any(np.asarray(b)), "nonzero biases not supported"

    x = np.asarray(x, np.float32)
    xT_pad = np.zeros((P, NP_), np.float32)
    xT_pad[:, :N] = x.T
    xT_pad = xT_pad.astype(bf16)
    cores, Clo, Chi, off, NCH = _prep(np.asarray(ei), np.asarray(ea))

    key = (NCH, tuple(Clo), tuple(Chi))
    if key not in _CACHE:
        _CACHE[key] = _build(Clo, Chi, off, NCH)
    nc = _CACHE[key]

    def cat(a, b_):
        return np.ascontiguousarray(np.concatenate(
            [np.asarray(a, np.float32), np.asarray(b_, np.float32)],
            axis=1).astype(bf16))

    iota = np.broadcast_to(np.arange(P, dtype=np.float32), (P, P))
    iotaP = np.ascontiguousarray(iota.astype(bf16))
    identP = np.ascontiguousarray(np.eye(P, dtype=np.float32))

    Wkv1 = cat(Wk1, Wv1)
    Wqs1 = cat(Wq1, Ws1)
    Wkv2 = cat(Wk2, Wv2)
    Wqs2 = cat(Wq2, Ws2)
    We1b = np.ascontiguousarray(np.asarray(We1, np.float32).astype(bf16))
    We2b = np.ascontiguousarray(np.asarray(We2, np.float32).astype(bf16))

    in_maps = []
    for c in range(NCORES):
        pc = cores[c]
        in_maps.append({
            "xT_full": xT_pad,
            "xT_loc": np.ascontiguousarray(xT_pad[:, c * NLC:(c + 1) * NLC]),
            "idx16": pc["idx16"], "drelT": pc["drelT"], "eaT": pc["eaT"],
            "iotaP": iotaP, "identP": identP,
            "Wkv1": Wkv1, "Wqs1": Wqs1, "We1": We1b,
            "Wkv2": Wkv2, "Wqs2": Wqs2, "We2": We2b,
        })
    res = run_bass_kernel_spmd(nc, in_maps, list(range(NCORES)))
    global LAST_RESULT, LAST_NC, LAST_IN_MAPS
    LAST_RESULT = res
    LAST_NC = nc
    LAST_IN_MAPS = in_maps
    out = np.concatenate([res.results[c]["out"] for c in range(NCORES)], axis=0)
    return np.ascontiguousarray(out[:N].astype(np.float32))


LAST_RESULT = None
LAST_NC = None
LAST_IN_MAPS = None
